# revision 9
# baseline (speedup 1.0000x reference)
import os
import sys

sys.path.insert(0, "/opt/trn_rl_repo")
import numpy as np
import concourse.bacc as bacc
import concourse.mybir as mybir
import concourse.tile as tile
from concourse import bass_utils

# DAS beamforming: image[b,p] = sum_s data[b,s,t[s,p]], then per-batch min-max norm.
#
# Sharding: 16 sensors per core (all batches on every core). Per-core partial
# images are ReduceScatter'd (batch row b lands on core b), each core
# normalizes its own batch row, host stacks the 8 rows.
#
# The gather runs as 5 large ap_gather calls (num_idxs is capped at ~32704 by
# the 64KB GPSIMD scratch); each call covers a pixel range for both 8-sensor
# row-sets (A rows 0-7, B rows 8-15 of each 16-partition group) with the two
# index streams concatenated along the free axis.
N_CORES = 8
B, S, T = 8, 128, 4096
NX = NY = 256
P = NX * NY
DX = DY = 1e-3
VS, DT = 1550.0, 1e-7

# pixel chunk bounds per gather call (512-aligned, num_idxs = 2*W <= 32704)
BOUNDS = [0, 13312, 26624, 39936, 53248, 65536]
NCALL = len(BOUNDS) - 1
IMG_PX = 2048  # pixels per partial-write DMA

LAST_EXEC_NS = None
_NC = None


def _maybe_install_profhook():
    try:
        import types
        if "antenv.axon_hooks" not in sys.modules:
            import antenv
            mod = types.ModuleType("antenv.axon_hooks")
            _h = [None]
            mod.set_axon_ntff_profile_hook = lambda h: _h.__setitem__(0, h)
            mod.get_axon_ntff_profile_hook = lambda: _h[0]
            sys.modules["antenv.axon_hooks"] = mod
            antenv.axon_hooks = mod
            from trn_agent_boot.trn_boot import _ntff_profile_via_ctypes
            h = _ntff_profile_via_ctypes("/opt/axon/libaxon_pjrt.so")
            if h is not None:
                mod.set_axon_ntff_profile_hook(h)
    except Exception:
        pass


def _build():
    nc = bacc.Bacc("TRN2", target_bir_lowering=False, debug=False, num_devices=N_CORES)
    f32, f16, i16 = mybir.dt.float32, mybir.dt.float16, mybir.dt.int16
    AX, OP = mybir.AxisListType, mybir.AluOpType

    tab_d = nc.dram_tensor("tab", [128, T], f32, kind="ExternalInput")
    idx_d = []
    for m in range(NCALL):
        w = BOUNDS[m + 1] - BOUNDS[m]
        idx_d.append(
            nc.dram_tensor(f"idx{m}", [128, 2 * w // 16], i16, kind="ExternalInput")
        )
    maska_d = nc.dram_tensor("maska", [128, 8], f16, kind="ExternalInput")
    maskb_d = nc.dram_tensor("maskb", [128, 8], f16, kind="ExternalInput")
    ones_d = nc.dram_tensor("ones", [1, 128], f32, kind="ExternalInput")
    y_d = nc.dram_tensor("y", [1, P], f32, kind="ExternalOutput")

    with tile.TileContext(nc) as tc:
        with (
            tc.tile_pool(name="const", bufs=1) as cpool,
            tc.tile_pool(name="g", bufs=1) as gpool,
            tc.tile_pool(name="c16", bufs=4) as hpool,
            tc.tile_pool(name="img", bufs=3) as ipool,
            tc.tile_pool(name="ps", bufs=6, space="PSUM") as pspool,
            tc.tile_pool(name="psb", bufs=1, space="PSUM") as psbpool,
            tc.tile_pool(name="norm", bufs=1) as npool,
            tc.tile_pool(name="dram", bufs=1, space="DRAM") as dpool,
        ):
            tab = cpool.tile([128, T], f32)
            nc.sync.dma_start(tab[:], tab_d[:])
            maska = cpool.tile([128, 8], f16)
            maskb = cpool.tile([128, 8], f16)
            ones = cpool.tile([1, 128], f32)
            nc.scalar.dma_start(maska[:], maska_d[:])
            nc.scalar.dma_start(maskb[:], maskb_d[:])
            nc.scalar.dma_start(ones[:], ones_d[:])
            idxs = []
            for m in range(NCALL):
                w = BOUNDS[m + 1] - BOUNDS[m]
                it = cpool.tile([128, 2 * w // 16], i16, name=f"idxt{m}")
                nc.scalar.dma_start(it[:], idx_d[m][:])
                idxs.append(it)

            partials = []
            reds = []
            for m in range(NCALL):
                w = BOUNDS[m + 1] - BOUNDS[m]
                partials.append(dpool.tile([8, w], f32, name=f"partial{m}"))
                reds.append(dpool.tile([1, w], f32, name=f"red{m}"))
            scr = dpool.tile([1, 256], f32)

            redsb = npool.tile([128, P // 128], f32)

            dma_engines = [nc.sync, nc.scalar]
            n_dma = 0

            def do_call(m):
                nonlocal n_dma
                w = BOUNDS[m + 1] - BOUNDS[m]
                g = gpool.tile([128, 2 * BOUNDS[1]], f32, tag="g")
                nc.gpsimd.ap_gather(
                    out_ap=g[:, : 2 * w], in_ap=tab[:], idxs_ap=idxs[m][:],
                    channels=128, num_elems=T, d=1, num_idxs=2 * w,
                )
                pos = 0
                while pos < w:
                    ipx = min(IMG_PX, w - pos)
                    img = ipool.tile([8, IMG_PX], f32, tag="img")
                    for off in range(0, ipx, 512):
                        sl_a = slice(pos + off, pos + off + 512)
                        sl_b = slice(w + pos + off, w + pos + off + 512)
                        a16 = hpool.tile([128, 512], f16, tag="h")
                        nc.vector.tensor_copy(a16[:], g[:, sl_a])
                        b16 = hpool.tile([128, 512], f16, tag="h")
                        nc.vector.tensor_copy(b16[:], g[:, sl_b])
                        ps = pspool.tile([128, 512], f32, tag="ps")
                        nc.tensor.matmul(ps[:8, :], lhsT=maska[:], rhs=a16[:],
                                         start=True, stop=False)
                        nc.tensor.matmul(ps[:8, :], lhsT=maskb[:], rhs=b16[:],
                                         start=False, stop=True)
                        nc.vector.tensor_copy(img[:, off:off + 512], ps[:8, :])
                    eng = dma_engines[n_dma % len(dma_engines)]
                    n_dma += 1
                    eng.dma_start(partials[m][:, pos:pos + ipx], img[:, :ipx])
                    pos += ipx

            def do_rs(m):
                nc.gpsimd.collective_compute(
                    "ReduceScatter", mybir.AluOpType.add,
                    replica_groups=[list(range(N_CORES))],
                    ins=[partials[m].opt()], outs=[reds[m].opt()],
                )
                # red row chunk -> [rows of 512] in redsb (pixel p = q*512 + f)
                lo, hi = BOUNDS[m] // 512, BOUNDS[m + 1] // 512
                nc.sync.dma_start(
                    redsb[lo:hi, :],
                    reds[m][:].rearrange("a (q f) -> (a q) f", f=512),
                )

            # gpsimd program order: g0, g1, RS0, g2, RS1, g3, RS2, g4, RS3, RS4
            do_call(0)
            do_call(1)
            do_rs(0)
            do_call(2)
            do_rs(1)
            do_call(3)
            do_rs(2)
            do_call(4)
            do_rs(3)
            do_rs(4)

            # min/max over this core's batch row, then (x - mn) / (mx - mn)
            mn = npool.tile([128, 1], f32)
            mx = npool.tile([128, 1], f32)
            nc.vector.tensor_reduce(mn[:], redsb[:], axis=AX.X, op=OP.min)
            nc.vector.tensor_reduce(mx[:], redsb[:], axis=AX.X, op=OP.max)
            nc.sync.dma_start(scr[:, 0:128].rearrange("a f -> (a f)"),
                              mn[:].rearrange("p f -> (p f)"))
            nc.sync.dma_start(scr[:, 128:256].rearrange("a f -> (a f)"),
                              mx[:].rearrange("p f -> (p f)"))
            r2 = npool.tile([1, 256], f32)
            nc.sync.dma_start(r2[:], scr[:])
            mm = npool.tile([1, 4], f32)
            nc.vector.tensor_reduce(mm[:, 0:1], r2[:, 0:128], axis=AX.X, op=OP.min)
            nc.vector.tensor_reduce(mm[:, 1:2], r2[:, 128:256], axis=AX.X, op=OP.max)
            nc.vector.tensor_tensor(out=mm[:, 2:3], in0=mm[:, 1:2], in1=mm[:, 0:1],
                                    op=OP.subtract)
            nc.vector.reciprocal(mm[:, 3:4], mm[:, 2:3])
            st = npool.tile([1, 2], f32)
            nc.vector.tensor_copy(st[:, 0:1], mm[:, 0:1])
            nc.vector.tensor_copy(st[:, 1:2], mm[:, 3:4])
            psb = psbpool.tile([128, 2], f32)
            nc.tensor.matmul(psb[:], lhsT=ones[:], rhs=st[:], start=True, stop=True)
            bc = npool.tile([128, 2], f32)
            nc.vector.tensor_copy(bc[:], psb[:])
            nc.vector.tensor_tensor(out=redsb[:], in0=redsb[:],
                                    in1=bc[:, 0:1].to_broadcast([128, P // 128]),
                                    op=OP.subtract)
            nc.vector.tensor_tensor(out=redsb[:], in0=redsb[:],
                                    in1=bc[:, 1:2].to_broadcast([128, P // 128]),
                                    op=OP.mult)
            y3 = y_d[:].rearrange("a (q f) -> a q f", q=128)
            nc.sync.dma_start(y3[0], redsb[:])
    nc.compile()
    return nc


def _host_tables(sensor_data, sensor_xy):
    """Exact fp32 replication of the reference index computation + packing."""
    sd = np.asarray(sensor_data, dtype=np.float32)
    xy = np.asarray(sensor_xy)
    ix = np.arange(NX, dtype=np.float32)
    iy = np.arange(NY, dtype=np.float32)
    x = xy[:, 0].astype(np.float32)[:, None, None]
    y = xy[:, 1].astype(np.float32)[:, None, None]
    dxf = (x - ix[None, :, None]) * np.float32(DX)
    dyf = (y - iy[None, None, :]) * np.float32(DY)
    dis = np.sqrt(dxf * dxf + dyf * dyf)
    t = ((dis / np.float32(VS)) / np.float32(DT)).astype(np.int32)
    t = t.reshape(S, P).astype(np.int16)  # all values < 4096

    maska = np.zeros((128, 8), np.float16)
    maskb = np.zeros((128, 8), np.float16)
    for k in range(8):
        for u in range(8):
            maska[16 * k + u, u] = 1.0
            maskb[16 * k + 8 + u, u] = 1.0
    ones = np.ones((1, 128), np.float32)

    in_maps = []
    for c in range(N_CORES):
        sa = 16 * c + np.arange(8)       # sensors on rows 0-7 of each group
        sb = 16 * c + 8 + np.arange(8)   # sensors on rows 8-15
        tab = np.zeros((128, T), np.float32)
        tab3 = tab.reshape(8, 16, T)
        tab3[:, 0:8, :] = sd[:, sa, :].transpose(1, 0, 2)   # [group, batch, T]
        tab3[:, 8:16, :] = sd[:, sb, :].transpose(1, 0, 2)
        m = {"tab": tab, "maska": maska, "maskb": maskb, "ones": ones}
        for mi in range(NCALL):
            lo, hi = BOUNDS[mi], BOUNDS[mi + 1]
            w = hi - lo
            # linear stream per group: A pixels then B pixels
            lin = np.concatenate([t[sa, lo:hi], t[sb, lo:hi]], axis=1)  # [8, 2w]
            idx = lin.reshape(8, 2 * w // 16, 16).transpose(0, 2, 1).reshape(
                128, 2 * w // 16)
            m[f"idx{mi}"] = np.ascontiguousarray(idx)
        in_maps.append(m)
    return in_maps


def kernel(sensor_data, sensor_xy):
    global _NC, LAST_EXEC_NS
    if os.environ.get("BASS_TRACE"):
        _maybe_install_profhook()
    if _NC is None:
        _NC = _build()
    in_maps = _host_tables(sensor_data, sensor_xy)
    res = bass_utils.run_bass_kernel_spmd(_NC, in_maps, core_ids=list(range(N_CORES)))
    LAST_EXEC_NS = res.exec_time_ns
    rows = [np.asarray(res.results[c]["y"], dtype=np.float32).reshape(P)
            for c in range(N_CORES)]
    return np.stack(rows, axis=0).reshape(B, NX, NY)


# revision 12
# speedup vs baseline: 1.0418x; 1.0418x over previous
import os
import sys

sys.path.insert(0, "/opt/trn_rl_repo")
import numpy as np
import concourse.bacc as bacc
import concourse.mybir as mybir
import concourse.tile as tile
from concourse import bass_utils

# DAS beamforming: image[b,p] = sum_s data[b,s,t[s,p]], then per-batch min-max norm.
#
# Sharding: 16 sensors per core (all batches on every core). Per-core partial
# images are ReduceScatter'd (batch row b lands on core b), each core
# normalizes its own batch row, host stacks the 8 rows.
#
# The gather runs as 5 large ap_gather calls (num_idxs is capped at ~32704 by
# the 64KB GPSIMD scratch); each call covers a pixel range for both 8-sensor
# row-sets (A rows 0-7, B rows 8-15 of each 16-partition group) with the two
# index streams concatenated along the free axis.
N_CORES = 8
B, S, T = 8, 128, 4096
NX = NY = 256
P = NX * NY
DX = DY = 1e-3
VS, DT = 1550.0, 1e-7

# pixel chunk bounds per gather call (512-aligned, num_idxs = 2*W <= 32704)
BOUNDS = [8192 * i for i in range(9)]
NCALL = len(BOUNDS) - 1
IMG_PX = 2048  # pixels per partial-write DMA

LAST_EXEC_NS = None
_NC = None


def _maybe_install_profhook():
    try:
        import types
        if "antenv.axon_hooks" not in sys.modules:
            import antenv
            mod = types.ModuleType("antenv.axon_hooks")
            _h = [None]
            mod.set_axon_ntff_profile_hook = lambda h: _h.__setitem__(0, h)
            mod.get_axon_ntff_profile_hook = lambda: _h[0]
            sys.modules["antenv.axon_hooks"] = mod
            antenv.axon_hooks = mod
            from trn_agent_boot.trn_boot import _ntff_profile_via_ctypes
            h = _ntff_profile_via_ctypes("/opt/axon/libaxon_pjrt.so")
            if h is not None:
                mod.set_axon_ntff_profile_hook(h)
    except Exception:
        pass


def _build():
    nc = bacc.Bacc("TRN2", target_bir_lowering=False, debug=False, num_devices=N_CORES)
    f32, f16, i16 = mybir.dt.float32, mybir.dt.float16, mybir.dt.int16
    AX, OP = mybir.AxisListType, mybir.AluOpType

    tab_d = nc.dram_tensor("tab", [128, T], f32, kind="ExternalInput")
    idx_d = []
    for m in range(NCALL):
        w = BOUNDS[m + 1] - BOUNDS[m]
        idx_d.append(
            nc.dram_tensor(f"idx{m}", [128, 2 * w // 16], i16, kind="ExternalInput")
        )
    maska_d = nc.dram_tensor("maska", [128, 8], f16, kind="ExternalInput")
    maskb_d = nc.dram_tensor("maskb", [128, 8], f16, kind="ExternalInput")
    ones_d = nc.dram_tensor("ones", [1, 128], f32, kind="ExternalInput")
    y_d = nc.dram_tensor("y", [1, P], f32, kind="ExternalOutput")

    with tile.TileContext(nc) as tc:
        with (
            tc.tile_pool(name="const", bufs=1) as cpool,
            tc.tile_pool(name="g", bufs=2) as gpool,
            tc.tile_pool(name="c16", bufs=4) as hpool,
            tc.tile_pool(name="img", bufs=3) as ipool,
            tc.tile_pool(name="ps", bufs=6, space="PSUM") as pspool,
            tc.tile_pool(name="psb", bufs=1, space="PSUM") as psbpool,
            tc.tile_pool(name="norm", bufs=1) as npool,
            tc.tile_pool(name="dram", bufs=1, space="DRAM") as dpool,
        ):
            tab = cpool.tile([128, T], f32)
            nc.sync.dma_start(tab[:], tab_d[:])
            maska = cpool.tile([128, 8], f16)
            maskb = cpool.tile([128, 8], f16)
            ones = cpool.tile([1, 128], f32)
            nc.scalar.dma_start(maska[:], maska_d[:])
            nc.scalar.dma_start(maskb[:], maskb_d[:])
            nc.scalar.dma_start(ones[:], ones_d[:])
            idxs = []
            for m in range(NCALL):
                w = BOUNDS[m + 1] - BOUNDS[m]
                it = cpool.tile([128, 2 * w // 16], i16, name=f"idxt{m}")
                nc.scalar.dma_start(it[:], idx_d[m][:])
                idxs.append(it)

            partials = []
            reds = []
            for m in range(NCALL):
                w = BOUNDS[m + 1] - BOUNDS[m]
                partials.append(dpool.tile([8, w], f32, name=f"partial{m}"))
                reds.append(dpool.tile([1, w], f32, name=f"red{m}"))
            scr = dpool.tile([1, 256], f32)

            redsb = npool.tile([128, P // 128], f32)

            dma_engines = [nc.sync, nc.scalar]
            n_dma = 0

            def do_call(m):
                nonlocal n_dma
                w = BOUNDS[m + 1] - BOUNDS[m]
                g = gpool.tile([128, 2 * BOUNDS[1]], f32, tag="g")
                nc.gpsimd.ap_gather(
                    out_ap=g[:, : 2 * w], in_ap=tab[:], idxs_ap=idxs[m][:],
                    channels=128, num_elems=T, d=1, num_idxs=2 * w,
                )
                pos = 0
                while pos < w:
                    ipx = min(IMG_PX, w - pos)
                    img = ipool.tile([8, IMG_PX], f32, tag="img")
                    for off in range(0, ipx, 512):
                        sl_a = slice(pos + off, pos + off + 512)
                        sl_b = slice(w + pos + off, w + pos + off + 512)
                        a16 = hpool.tile([128, 512], f16, tag="h")
                        nc.vector.tensor_copy(a16[:], g[:, sl_a])
                        b16 = hpool.tile([128, 512], f16, tag="h")
                        nc.vector.tensor_copy(b16[:], g[:, sl_b])
                        ps = pspool.tile([128, 512], f32, tag="ps")
                        nc.tensor.matmul(ps[:8, :], lhsT=maska[:], rhs=a16[:],
                                         start=True, stop=False)
                        nc.tensor.matmul(ps[:8, :], lhsT=maskb[:], rhs=b16[:],
                                         start=False, stop=True)
                        nc.vector.tensor_copy(img[:, off:off + 512], ps[:8, :])
                    eng = dma_engines[n_dma % len(dma_engines)]
                    n_dma += 1
                    eng.dma_start(partials[m][:, pos:pos + ipx], img[:, :ipx])
                    pos += ipx

            def do_rs(m):
                nc.gpsimd.collective_compute(
                    "ReduceScatter", mybir.AluOpType.add,
                    replica_groups=[list(range(N_CORES))],
                    ins=[partials[m].opt()], outs=[reds[m].opt()],
                )
                # red row chunk -> [rows of 512] in redsb (pixel p = q*512 + f)
                lo, hi = BOUNDS[m] // 512, BOUNDS[m + 1] // 512
                nc.sync.dma_start(
                    redsb[lo:hi, :],
                    reds[m][:].rearrange("a (q f) -> (a q) f", f=512),
                )

            # gpsimd order: g0, g1, RS0, g2, RS1, ... (RS k after call k+2,
            # so its SEQ-held wait never stalls the next gather dispatch)
            do_call(0)
            do_call(1)
            for m in range(2, NCALL):
                do_rs(m - 2)
                do_call(m)
            do_rs(NCALL - 2)
            do_rs(NCALL - 1)

            # min/max over this core's batch row, then (x - mn) / (mx - mn)
            mn = npool.tile([128, 1], f32)
            mx = npool.tile([128, 1], f32)
            nc.vector.tensor_reduce(mn[:], redsb[:], axis=AX.X, op=OP.min)
            nc.vector.tensor_reduce(mx[:], redsb[:], axis=AX.X, op=OP.max)
            nc.sync.dma_start(scr[:, 0:128].rearrange("a f -> (a f)"),
                              mn[:].rearrange("p f -> (p f)"))
            nc.sync.dma_start(scr[:, 128:256].rearrange("a f -> (a f)"),
                              mx[:].rearrange("p f -> (p f)"))
            r2 = npool.tile([1, 256], f32)
            nc.sync.dma_start(r2[:], scr[:])
            mm = npool.tile([1, 4], f32)
            nc.vector.tensor_reduce(mm[:, 0:1], r2[:, 0:128], axis=AX.X, op=OP.min)
            nc.vector.tensor_reduce(mm[:, 1:2], r2[:, 128:256], axis=AX.X, op=OP.max)
            nc.vector.tensor_tensor(out=mm[:, 2:3], in0=mm[:, 1:2], in1=mm[:, 0:1],
                                    op=OP.subtract)
            nc.vector.reciprocal(mm[:, 3:4], mm[:, 2:3])
            st = npool.tile([1, 2], f32)
            nc.vector.tensor_copy(st[:, 0:1], mm[:, 0:1])
            nc.vector.tensor_copy(st[:, 1:2], mm[:, 3:4])
            psb = psbpool.tile([128, 2], f32)
            nc.tensor.matmul(psb[:], lhsT=ones[:], rhs=st[:], start=True, stop=True)
            bc = npool.tile([128, 2], f32)
            nc.vector.tensor_copy(bc[:], psb[:])
            nc.vector.tensor_tensor(out=redsb[:], in0=redsb[:],
                                    in1=bc[:, 0:1].to_broadcast([128, P // 128]),
                                    op=OP.subtract)
            nc.vector.tensor_tensor(out=redsb[:], in0=redsb[:],
                                    in1=bc[:, 1:2].to_broadcast([128, P // 128]),
                                    op=OP.mult)
            y3 = y_d[:].rearrange("a (q f) -> a q f", q=128)
            nc.sync.dma_start(y3[0], redsb[:])
    nc.compile()
    return nc


def _host_tables(sensor_data, sensor_xy):
    """Exact fp32 replication of the reference index computation + packing."""
    sd = np.asarray(sensor_data, dtype=np.float32)
    xy = np.asarray(sensor_xy)
    ix = np.arange(NX, dtype=np.float32)
    iy = np.arange(NY, dtype=np.float32)
    x = xy[:, 0].astype(np.float32)[:, None, None]
    y = xy[:, 1].astype(np.float32)[:, None, None]
    dxf = (x - ix[None, :, None]) * np.float32(DX)
    dyf = (y - iy[None, None, :]) * np.float32(DY)
    dis = np.sqrt(dxf * dxf + dyf * dyf)
    t = ((dis / np.float32(VS)) / np.float32(DT)).astype(np.int32)
    t = t.reshape(S, P).astype(np.int16)  # all values < 4096

    maska = np.zeros((128, 8), np.float16)
    maskb = np.zeros((128, 8), np.float16)
    for k in range(8):
        for u in range(8):
            maska[16 * k + u, u] = 1.0
            maskb[16 * k + 8 + u, u] = 1.0
    ones = np.ones((1, 128), np.float32)

    in_maps = []
    for c in range(N_CORES):
        sa = 16 * c + np.arange(8)       # sensors on rows 0-7 of each group
        sb = 16 * c + 8 + np.arange(8)   # sensors on rows 8-15
        tab = np.zeros((128, T), np.float32)
        tab3 = tab.reshape(8, 16, T)
        tab3[:, 0:8, :] = sd[:, sa, :].transpose(1, 0, 2)   # [group, batch, T]
        tab3[:, 8:16, :] = sd[:, sb, :].transpose(1, 0, 2)
        m = {"tab": tab, "maska": maska, "maskb": maskb, "ones": ones}
        for mi in range(NCALL):
            lo, hi = BOUNDS[mi], BOUNDS[mi + 1]
            w = hi - lo
            # linear stream per group: A pixels then B pixels
            lin = np.concatenate([t[sa, lo:hi], t[sb, lo:hi]], axis=1)  # [8, 2w]
            idx = lin.reshape(8, 2 * w // 16, 16).transpose(0, 2, 1).reshape(
                128, 2 * w // 16)
            m[f"idx{mi}"] = np.ascontiguousarray(idx)
        in_maps.append(m)
    return in_maps


def kernel(sensor_data, sensor_xy):
    global _NC, LAST_EXEC_NS
    if os.environ.get("BASS_TRACE"):
        _maybe_install_profhook()
    if _NC is None:
        _NC = _build()
    in_maps = _host_tables(sensor_data, sensor_xy)
    res = bass_utils.run_bass_kernel_spmd(_NC, in_maps, core_ids=list(range(N_CORES)))
    LAST_EXEC_NS = res.exec_time_ns
    rows = [np.asarray(res.results[c]["y"], dtype=np.float32).reshape(P)
            for c in range(N_CORES)]
    return np.stack(rows, axis=0).reshape(B, NX, NY)
